# revision 2
# baseline (speedup 1.0000x reference)
# Trainium2 Bass kernel for nn_DepthCorr (SiamRPN-style depthwise correlation head).
#
# Pipeline (per batch):
#   kf   = relu(bn(conv3x3(kernel, Wk)))   [C=256, 7,7]  -> [H=256, 5,5]
#   sf   = relu(bn(conv3x3(search, Ws)))   [C=256,31,31] -> [H=256,29,29]
#   corr = relu(dwxcorr(sf, kf))                         -> [H=256,25,25]
#   out  = relu(bn(conv1x1(corr, Wf)))                   -> [C=256,25,25]
#
# The PE runs the dense convs (fp32r/fp16) plus 17 of the 32 depthwise-xcorr
# (batch, channel-group) units as diagonal-weight matmuls; the DVE runs the
# other 15 units as fused scalar_tensor_tensor chains (acc = sf*kf + acc,
# one 1x-mode op per tap) AND builds the PE units' diagonal weight tiles at
# 2x mode: diagT[c, m, t] = kfc[c, t] * eyeT[c, m, t] with a DMA'd
# replicated-eye so every operand is step-1 innermost. gpsimd does NOT run
# tensor ops at steady state: a gpsimd tensor op starves the DVE completely
# for its duration (shared SBUF port, measured), which is why the diag
# builds live on the DVE.
#
# Sharding: pure data-parallel over batch (128 batches / 8 cores).

import numpy as np
from contextlib import ExitStack

import concourse.bass as bass
import concourse.mybir as mybir
import concourse.tile as tile
from concourse import bacc
from concourse.bass_utils import run_bass_kernel_spmd

B, C, H = 128, 256, 256
N_CORES = 8
NB = B // N_CORES
EPS = 1e-5
FP = mybir.dt.float32
FR = mybir.dt.float32r
F16 = mybir.dt.float16
RELU = mybir.ActivationFunctionType.Relu
MULT = mybir.AluOpType.mult
ADD = mybir.AluOpType.add
IDENT = mybir.ActivationFunctionType.Copy

PE_UNITS = {(b, 1) for b in range(16) if b not in (5, 9, 13)} | {(15, 0)}
DELAY = 3  # conv3 for batch b is emitted after conv2 of batch b+DELAY
DIAG_AHEAD = 3


def _build_nc(nb=NB, pe_units=PE_UNITS, delay=DELAY):
    nc = bacc.Bacc()

    search = nc.declare_dram_parameter("search", [nb, C, 31, 32], F16, isOutput=False)
    kin = nc.declare_dram_parameter("kin", [128, 2, 7, 7, nb], F16, isOutput=False)
    wk_d = nc.declare_dram_parameter("wk", [128, 36, 128], F16, isOutput=False)
    ws_d = nc.declare_dram_parameter("ws", [128, 36, 128], F16, isOutput=False)
    wf_d = nc.declare_dram_parameter("wf", [128, 4, 128], F16, isOutput=False)
    bias_d = nc.declare_dram_parameter("bias", [128, 6], FP, isOutput=False)
    mask_d = nc.declare_dram_parameter("mask", [128, 128], F16, isOutput=False)
    out_d = nc.declare_dram_parameter("out", [nb, C, 25, 25], FP, isOutput=True)

    C2_SPLITS = [(0, 16), (16, 13)]  # conv2 output row splits (N = 464 / 377)
    XC_SPLITS = [(0, 13), (13, 12)]  # PE-xcorr output row splits
    O_SPLITS = [(0, 325), (325, 300)]  # conv3 over flat 625

    with tile.TileContext(nc) as tc, ExitStack() as ctx:
        wpool = ctx.enter_context(tc.tile_pool(name="wpool", bufs=1))
        kpool = ctx.enter_context(tc.tile_pool(name="kpool", bufs=1))
        spool = ctx.enter_context(tc.tile_pool(name="spool", bufs=3))
        fpool = ctx.enter_context(tc.tile_pool(name="fpool", bufs=delay + 1))
        apool = ctx.enter_context(tc.tile_pool(name="apool", bufs=2 * (delay + 1)))
        dpool = ctx.enter_context(tc.tile_pool(name="dpool", bufs=DIAG_AHEAD + 1))
        ppool = ctx.enter_context(tc.tile_pool(name="ppool", bufs=3))
        cpool = ctx.enter_context(tc.tile_pool(name="cpool", bufs=delay + 1))
        opool = ctx.enter_context(tc.tile_pool(name="opool", bufs=2))
        ps_c = ctx.enter_context(tc.tile_pool(name="ps_c", bufs=4, space="PSUM"))
        ps_x = ctx.enter_context(tc.tile_pool(name="ps_x", bufs=2, space="PSUM"))
        ps_o = ctx.enter_context(tc.tile_pool(name="ps_o", bufs=2, space="PSUM"))

        s_tiles = {}

        def load_search(b):
            s_sb = spool.tile([128, 2, 31, 32], F16, tag="sin")
            nc.sync.dma_start(out=s_sb[:, 0, :, :], in_=search[b, 0:128, :, :])
            nc.scalar.dma_start(out=s_sb[:, 1, :, :], in_=search[b, 128:256, :, :])
            s_tiles[b] = s_sb

        wk_sb = wpool.tile([128, 36, 128], F16, tag="wk")
        ws_sb = wpool.tile([128, 36, 128], F16, tag="ws")
        wf_sb = wpool.tile([128, 4, 128], F16, tag="wf")
        bias_sb = wpool.tile([128, 6], FP, tag="bias")
        mask_sb = wpool.tile([128, 128], F16, tag="mask")
        k_sbs = []
        for cg in range(2):
            k_sb = kpool.tile([128, 7, 7, nb], F16, tag=f"kin{cg}")
            k_sbs.append(k_sb)
        # kf_sb[c, hg, tap, b] fp32 — per-partition scalars for the DVE STT
        kf_sb = kpool.tile([128, 2, 25, nb], FP, tag="kf")
        # kfc_sb[c, hg, b, tap(26pad)] fp16 — step-1 taps for the diag build
        kfc_sb = kpool.tile([128, 2, nb, 26], F16, tag="kfc")

        # conv1's inputs + weights race ahead so kf (which gates every
        # xcorr unit) is ready ASAP; search[0] follows on the same queues.
        nc.scalar.dma_start(out=wk_sb[:], in_=wk_d[:])
        for cg in range(2):
            nc.sync.dma_start(out=k_sbs[cg][:], in_=kin[:, cg])
        nc.sync.dma_start(out=bias_sb[:], in_=bias_d[:])
        nc.gpsimd.dma_start(out=ws_sb[:], in_=ws_d[:])
        nc.gpsimd.dma_start(out=mask_sb[:], in_=mask_d[:])
        load_search(0)

        def load_deferred_consts():
            nc.gpsimd.dma_start(out=wf_sb[:], in_=wf_d[:])

        def conv1():
            for hg in range(2):
                ps = ps_c.tile([128, 5, 5, nb], FP, tag="psc")
                n_mm = 0
                for cg in range(2):
                    for dy in range(3):
                        for dx in range(3):
                            t = dy * 3 + dx
                            nc.tensor.matmul(
                                ps[:],
                                lhsT=wk_sb[:, hg * 18 + t * 2 + cg, :],
                                rhs=k_sbs[cg][:, dy:dy + 5, dx:dx + 5, :],
                                start=(n_mm == 0),
                                stop=(n_mm == 17),
                            )
                            n_mm += 1
                nc.scalar.activation(
                    out=kf_sb[:, hg, :, :],
                    in_=ps.rearrange("p a b c -> p (a b) c"),
                    func=RELU,
                    bias=bias_sb[:, 0 + hg:1 + hg],
                    scale=1.0,
                )
                nc.scalar.activation(
                    out=kfc_sb[:, hg, :, 0:25].rearrange("p b t -> p t b"),
                    in_=ps.rearrange("p a b c -> p (a b) c"),
                    func=RELU,
                    bias=bias_sb[:, 0 + hg:1 + hg],
                    scale=1.0,
                )

        def conv2(b):
            s_sb = s_tiles.pop(b)
            # sf: [c, hg, 29, 30] (col 29 garbage)
            sf_sb = fpool.tile([128, 2, 29, 30], F16, tag="sf")
            for hg in range(2):
                for (y0, ny) in C2_SPLITS:
                    ps = ps_c.tile([128, ny, 29], FP, tag="psc")
                    n_mm = 0
                    for cg in range(2):
                        for dy in range(3):
                            for dx in range(3):
                                t = dy * 3 + dx
                                nc.tensor.matmul(
                                    ps[:],
                                    lhsT=ws_sb[:, hg * 18 + t * 2 + cg, :],
                                    rhs=s_sb[
                                        :, cg, dy + y0:dy + y0 + ny, dx:dx + 29
                                    ],
                                    start=(n_mm == 0),
                                    stop=(n_mm == 17),
                                )
                                n_mm += 1
                    nc.scalar.activation(
                        out=sf_sb[:, hg, y0:y0 + ny, 0:29],
                        in_=ps[:],
                        func=RELU,
                        bias=bias_sb[:, 2 + hg:3 + hg],
                        scale=1.0,
                    )
            return sf_sb

        diag_tiles = {}

        def build_diag(b, hg):
            # DVE 1x: diag[c, t, m] = kf[c, t] * (c == m); m kept innermost
            # so the PE LDWEIGHTS stays contiguous (FWL) — a transposed
            # layout made LDW strided and 3x slower (measured v5).
            diag = dpool.tile([128, 25, 128], F16, tag="diag")
            nc.vector.tensor_tensor(
                out=diag[:],
                in0=kfc_sb[:, hg, b, 0:25].unsqueeze(2).broadcast_to(
                    [128, 25, 128]),
                in1=mask_sb.unsqueeze(1).broadcast_to([128, 25, 128]),
                op=MULT)
            diag_tiles[(b, hg)] = diag

        def xcorr_unit_pe(b, hg, sf_sb, corr_sb):
            diag = diag_tiles.pop((b, hg))
            for (y0, ny) in XC_SPLITS:
                ps = ps_x.tile([128, ny, 25], FP, tag="psx")
                n_mm = 0
                for ti in range(5):
                    for tj in range(5):
                        t = ti * 5 + tj
                        nc.tensor.matmul(
                            ps[:],
                            lhsT=diag[:, t, :],
                            rhs=sf_sb[:, hg, ti + y0:ti + y0 + ny,
                                      tj:tj + 25],
                            start=(n_mm == 0),
                            stop=(n_mm == 24),
                        )
                        n_mm += 1
                nc.scalar.activation(
                    out=corr_sb[:, hg, y0 * 25:(y0 + ny) * 25].rearrange(
                        "p (a c) -> p a c", c=25),
                    in_=ps[:],
                    func=RELU,
                    scale=1.0,
                )

        def xcorr_unit(b, hg, sf_sb, corr_sb):
            if (b, hg) in pe_units:
                xcorr_unit_pe(b, hg, sf_sb, corr_sb)
                return
            # hybrid: even taps are DVE STT chains (1x); odd taps are ACT
            # products (per-partition kf scale) + DVE tensor_tensor adds
            # (2x) — splits the MAC work across both engines.
            acc = apool.tile([128, 25, 25], F16, tag="acc")
            for t in range(25):
                i, j = divmod(t, 5)
                in0 = sf_sb[:, hg, i:i + 25, j:j + 25]
                sc = kf_sb[:, hg, t, b:b + 1]
                if t == 0:
                    nc.scalar.activation(
                        out=acc[:], in_=in0, func=IDENT, scale=sc)
                elif t % 2 == 1:
                    prod = ppool.tile([128, 25, 25], F16, tag="prod")
                    nc.scalar.activation(
                        out=prod[:], in_=in0, func=IDENT, scale=sc)
                    nc.vector.tensor_tensor(
                        out=acc[:], in0=acc[:], in1=prod[:], op=ADD)
                else:
                    nc.vector.scalar_tensor_tensor(
                        out=acc[:], in0=in0, scalar=sc, in1=acc[:],
                        op0=MULT, op1=ADD)
            nc.scalar.activation(
                out=corr_sb[:, hg, :].rearrange("p (a b) -> p a b", a=25),
                in_=acc[:], func=RELU, scale=1.0)

        def conv3(b, corr_sb):
            out_sb = opool.tile([128, 2, 625], FP, tag="osb")
            for og in range(2):
                for (x0, nx) in O_SPLITS:
                    ps = ps_o.tile([128, nx], FP, tag="pso")
                    for hg in range(2):
                        nc.tensor.matmul(
                            ps[:],
                            lhsT=wf_sb[:, hg * 2 + og, :],
                            rhs=corr_sb[:, hg, x0:x0 + nx],
                            start=(hg == 0),
                            stop=(hg == 1),
                        )
                    nc.scalar.activation(
                        out=out_sb[:, og, x0:x0 + nx],
                        in_=ps[:],
                        func=RELU,
                        bias=bias_sb[:, 4 + og:5 + og],
                        scale=1.0,
                    )
                q = nc.sync if og == 0 else nc.scalar
                q.dma_start(
                    out=out_d[b, og * 128:(og + 1) * 128, :, :].rearrange(
                        "c h w -> c (h w)"
                    ),
                    in_=out_sb[:, og, 0:625],
                )

        conv1()
        diag_order = sorted(pe_units)
        di = 0

        def pump_diags(upto):
            nonlocal di
            while di < len(diag_order) and di < upto:
                build_diag(*diag_order[di])
                di += 1

        corr_tiles = {}
        n_pe_done = 0
        for b in range(nb):
            if b + 1 < nb:
                load_search(b + 1)
            if b == 0:
                load_deferred_consts()
            pump_diags(n_pe_done + DIAG_AHEAD)
            sf_sb = conv2(b)
            corr_sb = cpool.tile([128, 2, 625], F16, tag="corr")
            corr_tiles[b] = corr_sb
            for hg in range(2):
                xcorr_unit(b, hg, sf_sb, corr_sb)
                if (b, hg) in pe_units:
                    n_pe_done += 1
            if b >= delay:
                conv3(b - delay, corr_tiles.pop(b - delay))
        for b in sorted(corr_tiles):
            conv3(b, corr_tiles[b])

    nc.compile()
    return nc


def _fold_bn(W, g, be, m, v):
    inv = (g.astype(np.float64) / np.sqrt(v.astype(np.float64) + EPS))
    Wp = (W.astype(np.float64) * inv[:, None, None, None]).astype(np.float32)
    bp = (be.astype(np.float64) - m.astype(np.float64) * inv).astype(np.float32)
    return Wp, bp


def _pack_weights(Wk, gk, bk, mk, vk, Ws, gs, bs, ms, vs, Wf, gf, bf, mf, vf):
    Wkp, bkp = _fold_bn(Wk, gk, bk, mk, vk)
    Wsp, bsp = _fold_bn(Ws, gs, bs, ms, vs)
    Wfp, bfp = _fold_bn(Wf, gf, bf, mf, vf)

    def pack33(Wp):  # [H, C, 3, 3] -> [k, (hg, t, cg), m]
        w = Wp.reshape(2, 128, 2, 128, 3, 3)
        w = w.transpose(3, 0, 4, 5, 2, 1)
        return np.ascontiguousarray(w.reshape(128, 36, 128))

    wk_h = pack33(Wkp).astype(np.float16)
    ws_h = pack33(Wsp).astype(np.float16)
    w = Wfp[:, :, 0, 0].reshape(2, 128, 2, 128)
    wf_h = np.ascontiguousarray(
        w.transpose(3, 2, 0, 1).reshape(128, 4, 128)).astype(np.float16)

    bias_h = np.zeros((128, 6), np.float32)
    bias_h[:, 0] = bkp[0:128]
    bias_h[:, 1] = bkp[128:256]
    bias_h[:, 2] = bsp[0:128]
    bias_h[:, 3] = bsp[128:256]
    bias_h[:, 4] = bfp[0:128]
    bias_h[:, 5] = bfp[128:256]

    mask_h = np.eye(128, dtype=np.float16)
    return wk_h, ws_h, wf_h, bias_h, mask_h


_NC_CACHE = {}


def _get_nc(nb):
    if nb not in _NC_CACHE:
        _NC_CACHE[nb] = _build_nc(nb)
    return _NC_CACHE[nb]


def run(inputs, trace=False):
    kernel = np.asarray(inputs["kernel"], np.float32)
    search = np.asarray(inputs["search"], np.float32)
    wk_h, ws_h, wf_h, bias_h, mask_h = _pack_weights(
        np.asarray(inputs["Wk"]), np.asarray(inputs["gk"]), np.asarray(inputs["bk"]),
        np.asarray(inputs["mk"]), np.asarray(inputs["vk"]),
        np.asarray(inputs["Ws"]), np.asarray(inputs["gs"]), np.asarray(inputs["bs"]),
        np.asarray(inputs["ms"]), np.asarray(inputs["vs"]),
        np.asarray(inputs["Wf"]), np.asarray(inputs["gf"]), np.asarray(inputs["bf"]),
        np.asarray(inputs["mf"]), np.asarray(inputs["vf"]),
    )
    nc = _get_nc(NB)
    search_p = np.zeros((B, C, 31, 32), np.float16)
    search_p[:, :, :, :31] = search
    in_maps = []
    for i in range(N_CORES):
        kk = kernel[i * NB:(i + 1) * NB].reshape(NB, 2, 128, 7, 7)
        kin_h = np.ascontiguousarray(kk.transpose(2, 1, 3, 4, 0)).astype(np.float16)
        in_maps.append({
            "search": np.ascontiguousarray(search_p[i * NB:(i + 1) * NB]),
            "kin": kin_h,
            "wk": wk_h, "ws": ws_h, "wf": wf_h, "bias": bias_h,
            "mask": mask_h,
        })
    res = run_bass_kernel_spmd(
        nc, in_maps, core_ids=list(range(N_CORES)), trace=trace
    )
    out = np.concatenate([res.results[i]["out"] for i in range(N_CORES)], axis=0)
    return out, res


def kernel(**inputs):
    out, _ = run(inputs, trace=False)
    return out


# revision 3
# speedup vs baseline: 1.0743x; 1.0743x over previous
# Trainium2 Bass kernel for nn_DepthCorr — v5.
#
# Pipeline (per batch):
#   kf   = relu(bn(conv3x3(kernel, Wk)))   [C=256, 7,7]  -> [H=256, 5,5]
#   sf   = relu(bn(conv3x3(search, Ws)))   [C=256,31,31] -> [H=256,29,29]
#   corr = relu(dwxcorr(sf, kf))                         -> [H=256,25,25]
#   out  = relu(bn(conv1x1(corr, Wf)))                   -> [C=256,25,25]
#
# The PE runs the dense convs (fp32r/fp16) plus 17 of the 32 depthwise-xcorr
# (batch, channel-group) units as diagonal-weight matmuls; the DVE runs the
# other 15 units as fused scalar_tensor_tensor chains (acc = sf*kf + acc,
# one 1x-mode op per tap) AND builds the PE units' diagonal weight tiles at
# 2x mode: diagT[c, m, t] = kfc[c, t] * eyeT[c, m, t] with a DMA'd
# replicated-eye so every operand is step-1 innermost. gpsimd does NOT run
# tensor ops at steady state: a gpsimd tensor op starves the DVE completely
# for its duration (shared SBUF port, measured), which is why the diag
# builds live on the DVE.
#
# Sharding: pure data-parallel over batch (128 batches / 8 cores).

import numpy as np
from contextlib import ExitStack

import concourse.bass as bass
import concourse.mybir as mybir
import concourse.tile as tile
from concourse import bacc
from concourse.bass_utils import run_bass_kernel_spmd

B, C, H = 128, 256, 256
N_CORES = 8
NB = B // N_CORES
EPS = 1e-5
FP = mybir.dt.float32
FR = mybir.dt.float32r
F16 = mybir.dt.float16
RELU = mybir.ActivationFunctionType.Relu
MULT = mybir.AluOpType.mult
ADD = mybir.AluOpType.add
IDENT = mybir.ActivationFunctionType.Copy

PE_UNITS = {(b, 1) for b in range(16) if b not in (3, 7, 11)} | {(15, 0)}
DELAY = 3  # conv3 for batch b is emitted after conv2 of batch b+DELAY
DIAG_AHEAD = 6


def _build_nc(nb=NB, pe_units=PE_UNITS, delay=DELAY):
    nc = bacc.Bacc()

    search = nc.declare_dram_parameter("search", [nb, C, 31, 32], F16, isOutput=False)
    kin = nc.declare_dram_parameter("kin", [128, 2, 7, 7, nb], F16, isOutput=False)
    wk_d = nc.declare_dram_parameter("wk", [128, 36, 128], F16, isOutput=False)
    ws_d = nc.declare_dram_parameter("ws", [128, 36, 128], F16, isOutput=False)
    wf_d = nc.declare_dram_parameter("wf", [128, 4, 128], F16, isOutput=False)
    bias_d = nc.declare_dram_parameter("bias", [128, 6], FP, isOutput=False)
    mask_d = nc.declare_dram_parameter("mask", [128, 128], F16, isOutput=False)
    out_d = nc.declare_dram_parameter("out", [nb, C, 25, 25], FP, isOutput=True)

    C2_SPLITS = [(0, 16), (16, 13)]  # conv2 output row splits (N = 464 / 377)
    XC_SPLITS = [(0, 13), (13, 12)]  # PE-xcorr output row splits
    O_SPLITS = [(0, 325), (325, 300)]  # conv3 over flat 625

    with tile.TileContext(nc) as tc, ExitStack() as ctx:
        wpool = ctx.enter_context(tc.tile_pool(name="wpool", bufs=1))
        kpool = ctx.enter_context(tc.tile_pool(name="kpool", bufs=1))
        spool = ctx.enter_context(tc.tile_pool(name="spool", bufs=3))
        fpool = ctx.enter_context(tc.tile_pool(name="fpool", bufs=delay + 1))
        apool = ctx.enter_context(tc.tile_pool(name="apool", bufs=2 * (delay + 1)))
        dpool = ctx.enter_context(tc.tile_pool(name="dpool", bufs=DIAG_AHEAD + 1))
        ppool = ctx.enter_context(tc.tile_pool(name="ppool", bufs=3))
        cpool = ctx.enter_context(tc.tile_pool(name="cpool", bufs=delay + 1))
        opool = ctx.enter_context(tc.tile_pool(name="opool", bufs=2))
        ps_c = ctx.enter_context(tc.tile_pool(name="ps_c", bufs=4, space="PSUM"))
        ps_x = ctx.enter_context(tc.tile_pool(name="ps_x", bufs=2, space="PSUM"))
        ps_o = ctx.enter_context(tc.tile_pool(name="ps_o", bufs=2, space="PSUM"))

        s_tiles = {}

        def load_search(b):
            s_sb = spool.tile([128, 2, 31, 32], F16, tag="sin")
            nc.sync.dma_start(out=s_sb[:, 0, :, :], in_=search[b, 0:128, :, :])
            nc.scalar.dma_start(out=s_sb[:, 1, :, :], in_=search[b, 128:256, :, :])
            s_tiles[b] = s_sb

        wk_sb = wpool.tile([128, 36, 128], F16, tag="wk")
        ws_sb = wpool.tile([128, 36, 128], F16, tag="ws")
        wf_sb = wpool.tile([128, 4, 128], F16, tag="wf")
        bias_sb = wpool.tile([128, 6], FP, tag="bias")
        mask_sb = wpool.tile([128, 128], F16, tag="mask")
        k_sbs = []
        for cg in range(2):
            k_sb = kpool.tile([128, 7, 7, nb], F16, tag=f"kin{cg}")
            k_sbs.append(k_sb)
        # kf_sb[c, hg, tap, b] fp32 — per-partition scalars for the DVE STT
        kf_sb = kpool.tile([128, 2, 25, nb], FP, tag="kf")
        # kfc_sb[c, hg, b, tap(26pad)] fp16 — step-1 taps for the diag build
        kfc_sb = kpool.tile([128, 2, nb, 26], F16, tag="kfc")

        # conv1's inputs + weights race ahead so kf (which gates every
        # xcorr unit) is ready ASAP; search[0] follows on the same queues.
        nc.scalar.dma_start(out=wk_sb[:], in_=wk_d[:])
        for cg in range(2):
            nc.sync.dma_start(out=k_sbs[cg][:], in_=kin[:, cg])
        nc.sync.dma_start(out=bias_sb[:], in_=bias_d[:])
        nc.gpsimd.dma_start(out=ws_sb[:], in_=ws_d[:])
        nc.gpsimd.dma_start(out=mask_sb[:], in_=mask_d[:])
        load_search(0)

        def load_deferred_consts():
            nc.gpsimd.dma_start(out=wf_sb[:], in_=wf_d[:])

        def conv1():
            for hg in range(2):
                ps = ps_c.tile([128, 5, 5, nb], FP, tag="psc")
                n_mm = 0
                for cg in range(2):
                    for dy in range(3):
                        for dx in range(3):
                            t = dy * 3 + dx
                            nc.tensor.matmul(
                                ps[:],
                                lhsT=wk_sb[:, hg * 18 + t * 2 + cg, :],
                                rhs=k_sbs[cg][:, dy:dy + 5, dx:dx + 5, :],
                                start=(n_mm == 0),
                                stop=(n_mm == 17),
                            )
                            n_mm += 1
                nc.scalar.activation(
                    out=kf_sb[:, hg, :, :],
                    in_=ps.rearrange("p a b c -> p (a b) c"),
                    func=RELU,
                    bias=bias_sb[:, 0 + hg:1 + hg],
                    scale=1.0,
                )
                nc.scalar.activation(
                    out=kfc_sb[:, hg, :, 0:25].rearrange("p b t -> p t b"),
                    in_=ps.rearrange("p a b c -> p (a b) c"),
                    func=RELU,
                    bias=bias_sb[:, 0 + hg:1 + hg],
                    scale=1.0,
                )

        def conv2(b):
            s_sb = s_tiles.pop(b)
            # sf: [c, hg, 29, 30] (col 29 garbage)
            sf_sb = fpool.tile([128, 2, 29, 30], F16, tag="sf")
            for hg in range(2):
                for (y0, ny) in C2_SPLITS:
                    ps = ps_c.tile([128, ny, 29], FP, tag="psc")
                    n_mm = 0
                    for cg in range(2):
                        for dy in range(3):
                            for dx in range(3):
                                t = dy * 3 + dx
                                nc.tensor.matmul(
                                    ps[:],
                                    lhsT=ws_sb[:, hg * 18 + t * 2 + cg, :],
                                    rhs=s_sb[
                                        :, cg, dy + y0:dy + y0 + ny, dx:dx + 29
                                    ],
                                    start=(n_mm == 0),
                                    stop=(n_mm == 17),
                                )
                                n_mm += 1
                    nc.scalar.activation(
                        out=sf_sb[:, hg, y0:y0 + ny, 0:29],
                        in_=ps[:],
                        func=RELU,
                        bias=bias_sb[:, 2 + hg:3 + hg],
                        scale=1.0,
                    )
            return sf_sb

        diag_tiles = {}

        def build_diag(b, hg):
            # DVE 1x: diag[c, t, m] = kf[c, t] * (c == m); m kept innermost
            # so the PE LDWEIGHTS stays contiguous (FWL) — a transposed
            # layout made LDW strided and 3x slower (measured v5).
            diag = dpool.tile([128, 25, 128], F16, tag="diag")
            nc.vector.tensor_tensor(
                out=diag[:],
                in0=kfc_sb[:, hg, b, 0:25].unsqueeze(2).broadcast_to(
                    [128, 25, 128]),
                in1=mask_sb.unsqueeze(1).broadcast_to([128, 25, 128]),
                op=MULT)
            diag_tiles[(b, hg)] = diag

        def xcorr_unit_pe(b, hg, sf_sb, corr_sb):
            diag = diag_tiles.pop((b, hg))
            for (y0, ny) in XC_SPLITS:
                ps = ps_x.tile([128, ny, 25], FP, tag="psx")
                n_mm = 0
                for ti in range(5):
                    for tj in range(5):
                        t = ti * 5 + tj
                        nc.tensor.matmul(
                            ps[:],
                            lhsT=diag[:, t, :],
                            rhs=sf_sb[:, hg, ti + y0:ti + y0 + ny,
                                      tj:tj + 25],
                            start=(n_mm == 0),
                            stop=(n_mm == 24),
                        )
                        n_mm += 1
                nc.scalar.activation(
                    out=corr_sb[:, hg, y0 * 25:(y0 + ny) * 25].rearrange(
                        "p (a c) -> p a c", c=25),
                    in_=ps[:],
                    func=RELU,
                    scale=1.0,
                )

        def xcorr_unit(b, hg, sf_sb, corr_sb):
            if (b, hg) in pe_units:
                xcorr_unit_pe(b, hg, sf_sb, corr_sb)
                return
            # hybrid: even taps are DVE STT chains (1x); odd taps are ACT
            # products (per-partition kf scale) + DVE tensor_tensor adds
            # (2x) — splits the MAC work across both engines.
            acc = apool.tile([128, 25, 25], F16, tag="acc")
            for t in range(25):
                i, j = divmod(t, 5)
                in0 = sf_sb[:, hg, i:i + 25, j:j + 25]
                sc = kf_sb[:, hg, t, b:b + 1]
                if t == 0:
                    nc.scalar.activation(
                        out=acc[:], in_=in0, func=IDENT, scale=sc)
                elif t % 2 == 1:
                    prod = ppool.tile([128, 25, 25], F16, tag="prod")
                    nc.scalar.activation(
                        out=prod[:], in_=in0, func=IDENT, scale=sc)
                    nc.vector.tensor_tensor(
                        out=acc[:], in0=acc[:], in1=prod[:], op=ADD)
                else:
                    nc.vector.scalar_tensor_tensor(
                        out=acc[:], in0=in0, scalar=sc, in1=acc[:],
                        op0=MULT, op1=ADD)
            nc.scalar.activation(
                out=corr_sb[:, hg, :].rearrange("p (a b) -> p a b", a=25),
                in_=acc[:], func=RELU, scale=1.0)

        def conv3(b, corr_sb):
            out_sb = opool.tile([128, 2, 625], FP, tag="osb")
            for og in range(2):
                for (x0, nx) in O_SPLITS:
                    ps = ps_o.tile([128, nx], FP, tag="pso")
                    for hg in range(2):
                        nc.tensor.matmul(
                            ps[:],
                            lhsT=wf_sb[:, hg * 2 + og, :],
                            rhs=corr_sb[:, hg, x0:x0 + nx],
                            start=(hg == 0),
                            stop=(hg == 1),
                        )
                    nc.scalar.activation(
                        out=out_sb[:, og, x0:x0 + nx],
                        in_=ps[:],
                        func=RELU,
                        bias=bias_sb[:, 4 + og:5 + og],
                        scale=1.0,
                    )
                q = nc.sync if og == 0 else nc.scalar
                q.dma_start(
                    out=out_d[b, og * 128:(og + 1) * 128, :, :].rearrange(
                        "c h w -> c (h w)"
                    ),
                    in_=out_sb[:, og, 0:625],
                )

        conv1()
        diag_order = sorted(pe_units)
        di = 0

        def pump_diags(upto):
            nonlocal di
            while di < len(diag_order) and di < upto:
                build_diag(*diag_order[di])
                di += 1

        corr_tiles = {}
        n_pe_done = 0
        for b in range(nb):
            if b + 1 < nb:
                load_search(b + 1)
            if b == 0:
                load_deferred_consts()
            pump_diags(n_pe_done + DIAG_AHEAD)
            sf_sb = conv2(b)
            corr_sb = cpool.tile([128, 2, 625], F16, tag="corr")
            corr_tiles[b] = corr_sb
            for hg in range(2):
                xcorr_unit(b, hg, sf_sb, corr_sb)
                if (b, hg) in pe_units:
                    n_pe_done += 1
            if b >= delay:
                conv3(b - delay, corr_tiles.pop(b - delay))
        for b in sorted(corr_tiles):
            conv3(b, corr_tiles[b])

    nc.compile()
    return nc


def _fold_bn(W, g, be, m, v):
    inv = (g.astype(np.float64) / np.sqrt(v.astype(np.float64) + EPS))
    Wp = (W.astype(np.float64) * inv[:, None, None, None]).astype(np.float32)
    bp = (be.astype(np.float64) - m.astype(np.float64) * inv).astype(np.float32)
    return Wp, bp


def _pack_weights(Wk, gk, bk, mk, vk, Ws, gs, bs, ms, vs, Wf, gf, bf, mf, vf):
    Wkp, bkp = _fold_bn(Wk, gk, bk, mk, vk)
    Wsp, bsp = _fold_bn(Ws, gs, bs, ms, vs)
    Wfp, bfp = _fold_bn(Wf, gf, bf, mf, vf)

    def pack33(Wp):  # [H, C, 3, 3] -> [k, (hg, t, cg), m]
        w = Wp.reshape(2, 128, 2, 128, 3, 3)
        w = w.transpose(3, 0, 4, 5, 2, 1)
        return np.ascontiguousarray(w.reshape(128, 36, 128))

    wk_h = pack33(Wkp).astype(np.float16)
    ws_h = pack33(Wsp).astype(np.float16)
    w = Wfp[:, :, 0, 0].reshape(2, 128, 2, 128)
    wf_h = np.ascontiguousarray(
        w.transpose(3, 2, 0, 1).reshape(128, 4, 128)).astype(np.float16)

    bias_h = np.zeros((128, 6), np.float32)
    bias_h[:, 0] = bkp[0:128]
    bias_h[:, 1] = bkp[128:256]
    bias_h[:, 2] = bsp[0:128]
    bias_h[:, 3] = bsp[128:256]
    bias_h[:, 4] = bfp[0:128]
    bias_h[:, 5] = bfp[128:256]

    mask_h = np.eye(128, dtype=np.float16)
    return wk_h, ws_h, wf_h, bias_h, mask_h


_NC_CACHE = {}


def _get_nc(nb):
    if nb not in _NC_CACHE:
        _NC_CACHE[nb] = _build_nc(nb)
    return _NC_CACHE[nb]


def run(inputs, trace=False):
    kernel = np.asarray(inputs["kernel"], np.float32)
    search = np.asarray(inputs["search"], np.float32)
    wk_h, ws_h, wf_h, bias_h, mask_h = _pack_weights(
        np.asarray(inputs["Wk"]), np.asarray(inputs["gk"]), np.asarray(inputs["bk"]),
        np.asarray(inputs["mk"]), np.asarray(inputs["vk"]),
        np.asarray(inputs["Ws"]), np.asarray(inputs["gs"]), np.asarray(inputs["bs"]),
        np.asarray(inputs["ms"]), np.asarray(inputs["vs"]),
        np.asarray(inputs["Wf"]), np.asarray(inputs["gf"]), np.asarray(inputs["bf"]),
        np.asarray(inputs["mf"]), np.asarray(inputs["vf"]),
    )
    nc = _get_nc(NB)
    search_p = np.zeros((B, C, 31, 32), np.float16)
    search_p[:, :, :, :31] = search
    in_maps = []
    for i in range(N_CORES):
        kk = kernel[i * NB:(i + 1) * NB].reshape(NB, 2, 128, 7, 7)
        kin_h = np.ascontiguousarray(kk.transpose(2, 1, 3, 4, 0)).astype(np.float16)
        in_maps.append({
            "search": np.ascontiguousarray(search_p[i * NB:(i + 1) * NB]),
            "kin": kin_h,
            "wk": wk_h, "ws": ws_h, "wf": wf_h, "bias": bias_h,
            "mask": mask_h,
        })
    res = run_bass_kernel_spmd(
        nc, in_maps, core_ids=list(range(N_CORES)), trace=trace
    )
    out = np.concatenate([res.results[i]["out"] for i in range(N_CORES)], axis=0)
    return out, res


def kernel(**inputs):
    out, _ = run(inputs, trace=False)
    return out
